# revision 10
# baseline (speedup 1.0000x reference)
"""Multi-head attention (B=16, N=1024, C=768, H=12) on 8 TRN2 NeuronCores.

Sharding: data-parallel over batch — each core runs the full attention block
for 2 of the 16 batch elements; weights are replicated, no collectives.

v2 design (engine-balanced, ~PE 255us / ACT 235us / DVE 130us per core):
  - QK^T: the two heads of a pair occupy disjoint PE row groups
    (tile_position (0,0)/(64,0)) and run CONCURRENTLY; each kT chunk
    stationary streams n=1024 (2x512 matmuls) -> full-array efficiency.
  - exp on ScalarE reads [128,1024] 2-bank PSUM tiles (halves the per-
    instruction 352-cycle overhead vs N=512).
  - AV: col-packed concurrent pair (h0 -> PSUM rows 0:64 via tile (0,0),
    h1 -> rows 64:128 via (0,64)) accumulating in ONE bank per n-half.
  - softmax denominators: 4 concurrent M=1 ones-matmuls (col groups
    0/32/64/96) accumulate sum_m e over the mc loop into one PSUM bank.
  - normalization: reciprocal_approx_fast (DVE), partition_broadcast
    (GpSimd - idle engine) to expand [1,512] -> [64,512], one DVE mul.
  - phase A (qkv projections) and phase C emission is interleaved into the
    attention loop as "filler" so the PE never idles while ScalarE chews
    through the 25M-element exp workload.

Container-specific findings baked in: (1) this walrus accepts at most ONE
semaphore wait per instruction - excess waits from the Tile scheduler are
hoisted onto injected EventSemaphore instructions in the BIR JSON.
"""

import json
from collections import deque

import numpy as np
import ml_dtypes
from contextlib import ExitStack

import concourse.bass as bass
import concourse.tile as tile
import concourse.bass2jax as b2j
import concourse.bass_utils as bu
from concourse import mybir
from concourse.bass_utils import run_bass_kernel_spmd

N_CORES = 8

# ---------------------------------------------------------------------------
# walrus single-wait workaround
# ---------------------------------------------------------------------------
_MAX_WAITS = 1
_orig_compile = bu.compile_bir_kernel


def _split_waits(bir_json: bytes) -> bytes:
    d = json.loads(bir_json)
    for f in d.get("functions", []):
        for blk in f.get("blocks", []):
            new_insts = []
            for inst in blk.get("instructions", []):
                si = inst.get("sync_info")
                waits = si.get("on_wait", []) if si else []
                if len(waits) > _MAX_WAITS:
                    extra, keep = waits[:-_MAX_WAITS], waits[-_MAX_WAITS:]
                    for ci in range(0, len(extra), _MAX_WAITS):
                        new_insts.append({
                            "debug": inst.get("debug", 0),
                            "engine": inst["engine"],
                            "ins": [],
                            "name": f"{inst['name']}-wsplit{ci}",
                            "opcode": "EventSemaphore",
                            "outs": [],
                            "sync_info": {
                                "on_update": [],
                                "on_wait": extra[ci:ci + _MAX_WAITS],
                            },
                        })
                    si["on_wait"] = keep
                new_insts.append(inst)
            blk["instructions"] = new_insts
    return json.dumps(d).encode()


def _patched_compile(bir_json, tmpdir, neff_name="file.neff"):
    return _orig_compile(_split_waits(bir_json), tmpdir, neff_name=neff_name)


def _install_patch():
    bu.compile_bir_kernel = _patched_compile
    b2j.compile_bir_kernel = _patched_compile


F32 = mybir.dt.float32
BF16 = mybir.dt.bfloat16

DIM = 768
NH = 12
HD = 64
SCALE = HD ** -0.5
NB = 2
N = 1024
NT = NB * N
NCC = DIM // 128
NHP = NH // 2


def build_attention_nc(reps: int = 1):
    nc = bass.Bass("TRN2", target_bir_lowering=False, debug=False)
    xT = nc.declare_dram_parameter("xT", [DIM, NT], BF16, isOutput=False)
    wqkvT = nc.declare_dram_parameter("wqkvT", [DIM, 3 * DIM], BF16, isOutput=False)
    wprojT = nc.declare_dram_parameter("wprojT", [DIM, DIM], BF16, isOutput=False)
    bias = nc.declare_dram_parameter("bias", [DIM, 1], F32, isOutput=False)
    out = nc.declare_dram_parameter("out", [DIM, NT], F32, isOutput=True)

    with tile.TileContext(nc) as tc:
        for rep in range(reps):
            _emit(nc, tc, xT, wqkvT, wprojT, bias, out, rep)
    return nc


def _emit(nc, tc, xT, wqkvT, wprojT, bias, out, rep):
    R = f"r{rep}_"
    with ExitStack() as ctx:
        p_const = ctx.enter_context(tc.tile_pool(name=R + "const", bufs=1))
        p_w = ctx.enter_context(tc.tile_pool(name=R + "w", bufs=1))
        p_qk = ctx.enter_context(tc.tile_pool(name=R + "qk", bufs=1))
        p_vp = ctx.enter_context(tc.tile_pool(name=R + "vp", bufs=1))
        p_aT = ctx.enter_context(tc.tile_pool(name=R + "aT", bufs=1))
        p_e = ctx.enter_context(tc.tile_pool(name=R + "E", bufs=6))
        p_rs = ctx.enter_context(tc.tile_pool(name=R + "rs", bufs=2))
        p_bc = ctx.enter_context(tc.tile_pool(name=R + "bc", bufs=3))
        p_ob = ctx.enter_context(tc.tile_pool(name=R + "ob", bufs=3))
        # PSUM pools: psF 1 (qkproj/V/C filler) + psS 4 + pa 2 + psD 1 = 8
        p_psF = ctx.enter_context(
            tc.tile_pool(name=R + "psF", bufs=1, space="PSUM"))
        p_psS = ctx.enter_context(
            tc.tile_pool(name=R + "psS", bufs=2, space="PSUM"))
        p_pa = ctx.enter_context(
            tc.tile_pool(name=R + "pa", bufs=2, space="PSUM"))
        p_psD = ctx.enter_context(
            tc.tile_pool(name=R + "psD", bufs=1, space="PSUM"))

        # ---- constants / weights / inputs ----
        bias_sb = []
        for oc in range(NCC):
            tbs = p_const.tile([128, 1], F32, name=R + f"bias_sb{oc}")
            nc.sync.dma_start(tbs[:], bias[oc * 128:(oc + 1) * 128, :])
            bias_sb.append(tbs)
        ones_bf = p_const.tile([128, 1], BF16, name=R + "ones_bf")
        nc.vector.memset(ones_bf[:], 1.0)
        ones_f32 = p_const.tile([128, 64], F32, name=R + "ones_f32")
        nc.vector.memset(ones_f32[:], 1.0)

        wq_t = []
        for c in range(NCC):
            t = p_w.tile([128, 3 * DIM], BF16, name=R + f"wq{c}")
            nc.sync.dma_start(t[:], wqkvT[c * 128:(c + 1) * 128, :])
            wq_t.append(t)
        wp_t = []
        for hp in range(NHP):
            t = p_w.tile([128, DIM], BF16, name=R + f"wp{hp}")
            nc.sync.dma_start(t[:], wprojT[hp * 128:(hp + 1) * 128, :])
            wp_t.append(t)
        xb = []
        for c in range(NCC):
            t = p_w.tile([128, NT], BF16, name=R + f"xb{c}")
            nc.sync.dma_start(t[:], xT[c * 128:(c + 1) * 128, :])
            xb.append(t)

        qT_t = [p_qk.tile([128, NT], BF16, name=R + f"qT{i}") for i in range(NHP)]
        kT_t = [p_qk.tile([128, NT], BF16, name=R + f"kT{i}") for i in range(NHP)]
        # token-major V: 16 tiles [128 tokens, 12 heads * 64]
        vp_t = [p_vp.tile([128, NH * HD], BF16, name=R + f"vp{i}")
                for i in range(NT // 128)]
        aT_t = {}

        # ---- emission units for projections (phase A) ----
        def emit_vp_half(nn, lo):
            """V projection for token tile nn, v-out cols [lo, lo+512) of 768
            (second call covers 256 cols). Output token-major into vp_t."""
            w = min(512, DIM - lo)
            ps = p_psF.tile([128, 512], F32, tag=R + "psF",
                            name=R + f"psv{nn}_{lo}")
            for c in range(NCC):
                nc.tensor.matmul(
                    ps[:, 0:w],
                    xb[c][:, nn * 128:(nn + 1) * 128],
                    wq_t[c][:, 2 * DIM + lo:2 * DIM + lo + w],
                    start=(c == 0), stop=(c == NCC - 1),
                )
            nc.vector.tensor_copy(vp_t[nn][:, lo:lo + w], ps[:, 0:w])

        def emit_qkproj_unit(hp, which, half4):
            """One quarter (512 tokens) of one 128-out chunk of Q^T or K^T."""
            wcol = (which * NHP + hp) * 128  # q chunks 0..5, k chunks 6..11
            dst = qT_t[hp] if which == 0 else kT_t[hp]
            ps = p_psF.tile([128, 512], F32, tag=R + "psF",
                            name=R + f"psq{hp}_{which}_{half4}")
            for c in range(NCC):
                nc.tensor.matmul(
                    ps[:],
                    wq_t[c][:, wcol:wcol + 128],
                    xb[c][:, half4 * 512:(half4 + 1) * 512],
                    start=(c == 0), stop=(c == NCC - 1),
                )
            nc.vector.tensor_copy(dst[:, half4 * 512:(half4 + 1) * 512], ps[:])

        def emit_c_unit(oc, b, nh):
            """Output projection for out-chunk oc, batch b, n-half nh."""
            ps = p_psF.tile([128, 512], F32, tag=R + "psF",
                            name=R + f"psP{oc}_{b}_{nh}")
            for cp in range(NHP):
                nc.tensor.matmul(
                    ps[:], wp_t[cp][:, oc * 128:(oc + 1) * 128],
                    aT_t[(b, cp)][:, nh * 512:(nh + 1) * 512],
                    start=(cp == 0), stop=(cp == NHP - 1))
            ob = p_ob.tile([128, 512], F32, tag=R + "ob")
            nc.vector.tensor_scalar_add(ob[:], ps[:], bias_sb[oc][:])
            nc.sync.dma_start(
                out[oc * 128:(oc + 1) * 128,
                    b * N + nh * 512:b * N + (nh + 1) * 512], ob[:])

        # ---- lead-in: qk projection for hp=0, first V tiles ----
        for which in range(2):
            for half4 in range(4):
                emit_qkproj_unit(0, which, half4)
        for nn in (0, 1):
            emit_vp_half(nn, 0)
            emit_vp_half(nn, 512)

        # ---- filler queue: remaining phase-A work, deadline-ordered.
        # Emission order IS the dependency order: every unit must be emitted
        # before the B-loop step that consumes its output (2 units drain per
        # mc step => unit at queue position q lands at step ~q/2).
        fillers = deque()
        for nn in range(2, 16):                     # vp b=0 rest, then b=1
            fillers.append((lambda nn=nn: emit_vp_half(nn, 0)))
            fillers.append((lambda nn=nn: emit_vp_half(nn, 512)))
        for hp_ in range(1, NHP):                   # qk proj hp=1..5
            for h4pair in range(2):                 # b=0 halves first
                for which in range(2):
                    for half4 in (h4pair * 2, h4pair * 2 + 1):
                        fillers.append(
                            (lambda p=hp_, w=which, h4=half4:
                             emit_qkproj_unit(p, w, h4)))

        # ---- attention (phase B), hp-major; fillers drip in per mc step ----
        for hp in range(NHP):
            for b in range(NB):
                boff = b * N
                at = p_aT.tile([128, N], BF16, name=R + f"aT{b}_{hp}")
                pa = [p_pa.tile([128, 512], F32, tag=R + "pa",
                                name=R + f"pa{b}_{hp}_{nh}")
                      for nh in range(2)]
                psD = p_psD.tile([128, 512], F32, tag=R + "psD",
                                 name=R + f"psD{b}_{hp}")
                for mc in range(N // 128):
                    first, last = (mc == 0), (mc == N // 128 - 1)
                    e_h = []
                    for hi in range(2):
                        pb_ = hi * 64
                        ps = p_psS.tile([128, 1024], F32, tag=R + "psS",
                                        name=R + f"psS{b}_{hp}_{mc}_{hi}")
                        for nh in range(2):
                            nc.tensor.matmul(
                                ps[:, nh * 512:(nh + 1) * 512],
                                kT_t[hp][pb_:pb_ + 64,
                                         boff + mc * 128:boff + (mc + 1) * 128],
                                qT_t[hp][pb_:pb_ + 64,
                                         boff + nh * 512:boff + (nh + 1) * 512],
                                start=True, stop=True,
                                tile_position=(pb_, 0),
                            )
                        e = p_e.tile([128, 1024], BF16, tag=R + "E")
                        nc.scalar.activation(
                            e[:], ps[:],
                            mybir.ActivationFunctionType.Exp,
                            scale=SCALE,
                        )
                        e_h.append(e)
                    for nh in range(2):
                        for hi in range(2):
                            nc.tensor.matmul(
                                pa[nh][hi * 64:(hi + 1) * 64, :],
                                vp_t[(boff + mc * 128) // 128][
                                    :, (2 * hp + hi) * HD:(2 * hp + hi + 1) * HD],
                                e_h[hi][:, nh * 512:(nh + 1) * 512],
                                start=first, stop=last,
                                tile_position=(0, hi * 64),
                                skip_group_check=True,
                            )
                    for hi in range(2):
                        for nh in range(2):
                            p = hi * 64 + nh * 32
                            nc.tensor.matmul(
                                psD[p:p + 1, :],
                                ones_bf[:, 0:1],
                                e_h[hi][:, nh * 512:(nh + 1) * 512],
                                start=first, stop=last,
                                tile_position=(0, p),
                                skip_group_check=True,
                            )
                    for _ in range(2):
                        if fillers:
                            fillers.popleft()()
                # epilogue: recip + matmul-broadcast + normalize
                rs = p_rs.tile([128, 512], F32, tag=R + "rs")
                for hi in range(2):
                    for nh in range(2):
                        p = hi * 64 + nh * 32
                        nc.vector.reciprocal(rs[p:p + 1, :], psD[p:p + 1, :])
                for nh in range(2):
                    psB = p_psF.tile([128, 512], F32, tag=R + "psF",
                                     name=R + f"psB{b}_{hp}_{nh}")
                    for hi in range(2):
                        p = hi * 64 + nh * 32
                        nc.tensor.matmul(
                            psB[hi * 64:(hi + 1) * 64, :],
                            ones_f32[p:p + 1, 0:64],
                            rs[p:p + 1, :],
                            start=True, stop=True,
                            tile_position=(p, hi * 64),
                            skip_group_check=True,
                        )
                    bc = p_bc.tile([128, 512], F32, tag=R + "bc")
                    nc.vector.tensor_copy(bc[:], psB[:])
                    nc.vector.tensor_mul(
                        at[:, nh * 512:(nh + 1) * 512], pa[nh][:], bc[:])
                aT_t[(b, hp)] = at

        while fillers:
            fillers.popleft()()

        # ---- phase C: transposed output projection ----
        for oc in range(NCC):
            for b in range(NB):
                for nh in range(2):
                    emit_c_unit(oc, b, nh)


# ---------------------------------------------------------------------------
# host wrapper
# ---------------------------------------------------------------------------
_CACHE = {}


def _prep_in_maps(x, w_qkv, w_proj, b_proj):
    x = np.asarray(x, dtype=np.float32)
    wqkvT = np.ascontiguousarray(np.asarray(w_qkv, dtype=np.float32).T
                                 ).astype(ml_dtypes.bfloat16)
    wprojT = np.ascontiguousarray(np.asarray(w_proj, dtype=np.float32).T
                                  ).astype(ml_dtypes.bfloat16)
    bias = np.asarray(b_proj, dtype=np.float32).reshape(DIM, 1).copy()
    in_maps = []
    for c in range(N_CORES):
        xs = x[c * NB:(c + 1) * NB]                       # [2, 1024, 768]
        xT = np.ascontiguousarray(xs.transpose(2, 0, 1).reshape(DIM, NT))
        in_maps.append({
            "xT": xT.astype(ml_dtypes.bfloat16),
            "wqkvT": wqkvT,
            "wprojT": wprojT,
            "bias": bias,
        })
    return in_maps


def kernel(x, w_qkv, w_proj, b_proj):
    _install_patch()
    if "nc" not in _CACHE:
        _CACHE["nc"] = build_attention_nc(1)
    nc = _CACHE["nc"]
    in_maps = _prep_in_maps(x, w_qkv, w_proj, b_proj)
    res = run_bass_kernel_spmd(nc, in_maps, core_ids=list(range(N_CORES)))
    shards = []
    for c in range(N_CORES):
        oT = res.results[c]["out"]                        # [768, 2048]
        shards.append(oT.T.reshape(NB, N, DIM))
    return np.ascontiguousarray(
        np.concatenate(shards, axis=0)).astype(np.float32)
